# revision 21
# baseline (speedup 1.0000x reference)
"""Trainium2 Bass kernel for single-query attention over per-sample concepts.

    sab[b, k] = (query[b] . concept[b, k]) / sqrt(D)
    score     = softmax(sab, axis=-1)
    out[b]    = sum_k score[b, k] * concept[b, k]

Shapes: query [256, 1024] f32, concept [256, 2048, 1024] f32 -> out [256, 1024].

Sharding: pure data parallel, batch 256 split as 32 samples on each of 8
NeuronCores. The tolerance (2e-2) admits streaming concept in bf16, halving
HBM traffic to 128 MiB per core (measured end-to-end rel err ~3e-3). Host
converts to bf16 and pre-folds the 1/sqrt(D) scale into query.

Per-core dataflow (chunk = [128 part, 8, 1024] bf16 = 2 MiB, partition p
holds k-rows 8p..8p+7 of a 1024-row block; 2 chunks per sample). The score
pass (row-wise dot with q) is spread across engines so no single engine
exceeds the DMA budget; per chunk the 8 rows split as: a few multiplied by
DVE tensor_tensor at 2x (stride-0 broadcast q) and reduced on ACT
(activation Copy + accum_out), two multiplied on GPSIMD tensor_mul (also
ACT-reduced), and the rest fused multiply+reduce on DVE (STT). a
alternates 5/4 between chunks to balance ACT vs DVE.

PE specifics (HAM clock-gate management): the sample's 32 weighted-sum
matmuls are grouped by PSUM bank (16x acc_lo then 16x acc_hi) to avoid
psum-queue cycling, and a few dummy matmuls are emitted between samples so
the PE never idles past the HAM activity window (idle re-throttles the PE
clock to K=4/8). Sample tails (reciprocal, normalize, output DMA) are
emitted one sample late so they never stall the DVE/DMA queues behind the
PE's accumulation.

Query broadcast to 128 partitions is a stride-0-source HWDGE DMA on the
sync ring (reads the 2 KiB row 128x from HBM).
"""

import numpy as np
import ml_dtypes
from contextlib import ExitStack

import concourse.bacc as bacc
import concourse.tile as tile
from concourse import mybir
from concourse.bass_utils import run_bass_kernel_spmd

B, K, D = 256, 2048, 1024
NCORES = 8
BL = B // NCORES          # 32 samples per core
RPP = 8                   # k-rows per partition within a chunk
CH_ROWS = 128 * RPP       # 1024 k-rows per chunk
NCH = K // CH_ROWS        # 2 chunks per sample
SCALE = 1.0 / float(np.sqrt(D))

A_PER_CHUNK = (5, 4)      # ACT-reduced rows for even/odd chunks
POOL_MULT = 0             # of the ACT set, rows multiplied on GPSIMD (shares SBUF ports with DVE; keep 0)
N_DUMMY = 5               # PE keep-warm matmuls between samples

_cache = {}


def build_nc(bl=BL):
    nc = bacc.Bacc("TRN2", target_bir_lowering=False, debug=False,
                   num_devices=NCORES)
    f32 = mybir.dt.float32
    bf16 = mybir.dt.bfloat16

    q = nc.dram_tensor("query", [bl, D], bf16, kind="ExternalInput")
    c = nc.dram_tensor("concept", [bl, NCH, 128, RPP, D], bf16,
                       kind="ExternalInput")
    out = nc.dram_tensor("out", [bl, D], f32, kind="ExternalOutput")

    with tile.TileContext(nc) as tc, ExitStack() as ctx:
        cpool = ctx.enter_context(tc.tile_pool(name="c", bufs=6))
        prpool = ctx.enter_context(tc.tile_pool(name="pr", bufs=4))
        qpool = ctx.enter_context(tc.tile_pool(name="q", bufs=4))
        stpool = ctx.enter_context(tc.tile_pool(name="st", bufs=3))
        scpool = ctx.enter_context(tc.tile_pool(name="sc", bufs=3))
        ecpool = ctx.enter_context(tc.tile_pool(name="ec", bufs=3))
        erpool = ctx.enter_context(tc.tile_pool(name="er", bufs=3))
        onepool = ctx.enter_context(tc.tile_pool(name="one", bufs=1))
        opool = ctx.enter_context(tc.tile_pool(name="o", bufs=6))
        plo = ctx.enter_context(tc.tile_pool(name="plo", bufs=2, space="PSUM"))
        phi = ctx.enter_context(tc.tile_pool(name="phi", bufs=2, space="PSUM"))
        dpool = ctx.enter_context(tc.tile_pool(name="dn", bufs=2, space="PSUM"))
        jpool = ctx.enter_context(tc.tile_pool(name="jk", bufs=1, space="PSUM"))

        ones = onepool.tile([128, 1], f32)
        nc.vector.memset(ones[:], 1.0)
        ones_bf = onepool.tile([128, 1], bf16)
        nc.vector.memset(ones_bf[:], 1.0)
        warm = onepool.tile([128, 256], bf16)
        nc.vector.memset(warm[:], 0.0)
        jk = jpool.tile([1, 256], f32)

        tails = []  # (acc_lo, acc_hi, denom, b) pending finalization

        def emit_tail():
            acc_lo, acc_hi, denom, bb = tails.pop(0)
            recip = opool.tile([1, 1], f32, name="recip")
            nc.vector.reciprocal(recip[:], denom[:])
            orow = opool.tile([1, D], f32, name="orow")
            nc.scalar.activation(out=orow[:, 0:512], in_=acc_lo[:],
                                 func=mybir.ActivationFunctionType.Copy,
                                 scale=recip[:])
            nc.scalar.activation(out=orow[:, 512:1024], in_=acc_hi[:],
                                 func=mybir.ActivationFunctionType.Copy,
                                 scale=recip[:])
            nc.sync.dma_start(out=out[bb : bb + 1, :], in_=orow[:])

        for b in range(bl):
            qb = qpool.tile([128, D], bf16)
            nc.sync.dma_start(
                out=qb[:],
                in_=q[b : b + 1, :].broadcast_to([128, D]))

            scols = scpool.tile([128, NCH * RPP], f32)
            ecols = ecpool.tile([128, NCH * RPP], bf16)
            ered = erpool.tile([128, 1], f32)
            acc_lo = plo.tile([1, 512], f32)
            acc_hi = phi.tile([1, 512], f32)
            cts = []

            for h in range(NCH):
                a = A_PER_CHUNK[h % 2]
                if b % 3 == 0 and h == 0:
                    a = 4  # fine ACT->DVE rebalance (~19 of 512 slices)
                pm = POOL_MULT
                ct = cpool.tile([128, RPP, D], bf16, name="ct")
                dma_eng = nc.sync if h % 2 == 0 else nc.scalar
                dma_eng.dma_start(out=ct[:], in_=c[b, h])
                cts.append(ct)

                prod = prpool.tile([128, a, D], bf16, name="prod")
                if a - pm:
                    nc.vector.tensor_tensor(
                        out=prod[:, 0 : a - pm], in0=ct[:, 0 : a - pm],
                        in1=qb[:].unsqueeze(1).broadcast_to([128, a - pm, D]),
                        op=mybir.AluOpType.mult)
                for j in range(a - pm, a):
                    nc.gpsimd.tensor_mul(prod[:, j], ct[:, j], qb[:])

                for j in range(RPP):
                    col = h * RPP + j
                    acc = scols[:, col : col + 1]
                    if j < a:
                        sl = prod[:, j]
                        nc.scalar.activation(
                            out=sl, in_=sl,
                            func=mybir.ActivationFunctionType.Copy,
                            accum_out=acc,
                        )
                    else:
                        scr = stpool.tile([128, D], bf16, name="scr")
                        nc.vector.scalar_tensor_tensor(
                            out=scr[:],
                            in0=ct[:, j], scalar=1.0, in1=qb[:],
                            op0=mybir.AluOpType.mult,
                            op1=mybir.AluOpType.mult,
                            accum_out=acc,
                        )

                # PE keep-warm: dummy matmuls gated on this chunk's score
                # columns fire spread across the PE's inter-sample gap,
                # stopping the HAM activity monitor from re-throttling.
                for col in (h * RPP + 2, h * RPP + 4, h * RPP + 6):
                    nc.tensor.matmul(jk[:, 0:1], ones[:],
                                     scols[:, col : col + 1],
                                     start=True, stop=True,
                                     skip_group_check=True)

            nc.scalar.activation(
                out=ecols[:], in_=scols[:],
                func=mybir.ActivationFunctionType.Exp,
                accum_out=ered[:],
            )

            # weighted-sum matmuls, grouped by PSUM bank
            for acc, lo in ((acc_lo, 0), (acc_hi, 512)):
                for h in range(NCH):
                    for j in range(RPP):
                        seq = h * RPP + j
                        e = ecols[:, seq : seq + 1]
                        nc.tensor.matmul(acc[:], e,
                                         cts[h][:, j, lo : lo + 512],
                                         start=(seq == 0),
                                         stop=(seq == NCH * RPP - 1))

            denom = dpool.tile([1, 1], f32)
            nc.tensor.matmul(denom[:], ones[:], ered[:], start=True, stop=True)

            tails.append((acc_lo, acc_hi, denom, b))
            if len(tails) > 1:
                emit_tail()

        while tails:
            emit_tail()

    nc.compile()
    return nc


def _run(query, concept, trace=False, trace_kwargs=None):
    if "nc" not in _cache:
        _cache["nc"] = build_nc()
    nc = _cache["nc"]

    bf = ml_dtypes.bfloat16
    qs = (np.asarray(query, np.float32) * SCALE).astype(bf)
    cb = np.asarray(concept, np.float32).astype(bf)

    in_maps = []
    for i in range(NCORES):
        in_maps.append({
            "query": np.ascontiguousarray(qs[i * BL : (i + 1) * BL]),
            "concept": np.ascontiguousarray(
                cb[i * BL : (i + 1) * BL].reshape(BL, NCH, 128, RPP, D)),
        })
    res = run_bass_kernel_spmd(
        nc, in_maps, core_ids=list(range(NCORES)),
        trace=trace, **(trace_kwargs or {}),
    )
    out = np.concatenate([res.results[i]["out"] for i in range(NCORES)], axis=0)
    return out.astype(np.float32), res


def kernel(query: np.ndarray, concept: np.ndarray) -> np.ndarray:
    out, _ = _run(np.asarray(query, np.float32), np.asarray(concept, np.float32))
    return out


# revision 22
# speedup vs baseline: 1.1575x; 1.1575x over previous
"""Trainium2 Bass kernel for single-query attention over per-sample concepts.

    sab[b, k] = (query[b] . concept[b, k]) / sqrt(D)
    score     = softmax(sab, axis=-1)
    out[b]    = sum_k score[b, k] * concept[b, k]

Shapes: query [256, 1024] f32, concept [256, 2048, 1024] f32 -> out [256, 1024].

Sharding: pure data parallel, batch 256 split as 32 samples on each of 8
NeuronCores. The tolerance (2e-2) admits streaming concept in bf16, halving
HBM traffic to 128 MiB per core (measured end-to-end rel err ~3e-3). Host
converts to bf16 and pre-folds the 1/sqrt(D) scale into query.

Per-core dataflow (chunk = [128 part, 8, 1024] bf16 = 2 MiB, partition p
holds k-rows 8p..8p+7 of a 1024-row block; 2 chunks per sample). The score
pass (row-wise dot with q) is spread across engines so no single engine
exceeds the DMA budget; per chunk the 8 rows split as: a few multiplied by
DVE tensor_tensor at 2x (stride-0 broadcast q) and reduced on ACT
(activation Copy + accum_out), two multiplied on GPSIMD tensor_mul (also
ACT-reduced), and the rest fused multiply+reduce on DVE (STT). a
alternates 5/4 between chunks to balance ACT vs DVE.

PE specifics (HAM clock-gate management): the sample's 32 weighted-sum
matmuls are grouped by PSUM bank (16x acc_lo then 16x acc_hi) to avoid
psum-queue cycling, and a few dummy matmuls are emitted between samples so
the PE never idles past the HAM activity window (idle re-throttles the PE
clock to K=4/8). Sample tails (reciprocal, normalize, output DMA) are
emitted one sample late so they never stall the DVE/DMA queues behind the
PE's accumulation.

Query broadcast to 128 partitions is a stride-0-source HWDGE DMA on the
sync ring (reads the 2 KiB row 128x from HBM).
"""

import numpy as np
import ml_dtypes
from contextlib import ExitStack

import concourse.bacc as bacc
import concourse.tile as tile
from concourse import mybir
from concourse.bass_utils import run_bass_kernel_spmd

B, K, D = 256, 2048, 1024
NCORES = 8
BL = B // NCORES          # 32 samples per core
RPP = 8                   # k-rows per partition within a chunk
CH_ROWS = 128 * RPP       # 1024 k-rows per chunk
NCH = K // CH_ROWS        # 2 chunks per sample
SCALE = 1.0 / float(np.sqrt(D))

A_PER_CHUNK = (5, 4)      # ACT-reduced rows for even/odd chunks
POOL_MULT = 0             # of the ACT set, rows multiplied on GPSIMD (shares SBUF ports with DVE; keep 0)
N_DUMMY = 5               # PE keep-warm matmuls between samples

_cache = {}


def build_nc(bl=BL):
    nc = bacc.Bacc("TRN2", target_bir_lowering=False, debug=False,
                   num_devices=NCORES)
    f32 = mybir.dt.float32
    bf16 = mybir.dt.bfloat16

    q = nc.dram_tensor("query", [bl, D], bf16, kind="ExternalInput")
    c = nc.dram_tensor("concept", [bl, NCH, 128, RPP, D], bf16,
                       kind="ExternalInput")
    out = nc.dram_tensor("out", [bl, D], f32, kind="ExternalOutput")

    with tile.TileContext(nc) as tc, ExitStack() as ctx:
        cpool = ctx.enter_context(tc.tile_pool(name="c", bufs=6))
        prpool = ctx.enter_context(tc.tile_pool(name="pr", bufs=4))
        qpool = ctx.enter_context(tc.tile_pool(name="q", bufs=4))
        stpool = ctx.enter_context(tc.tile_pool(name="st", bufs=3))
        scpool = ctx.enter_context(tc.tile_pool(name="sc", bufs=3))
        ecpool = ctx.enter_context(tc.tile_pool(name="ec", bufs=3))
        erpool = ctx.enter_context(tc.tile_pool(name="er", bufs=3))
        onepool = ctx.enter_context(tc.tile_pool(name="one", bufs=1))
        opool = ctx.enter_context(tc.tile_pool(name="o", bufs=6))
        plo = ctx.enter_context(tc.tile_pool(name="plo", bufs=2, space="PSUM"))
        phi = ctx.enter_context(tc.tile_pool(name="phi", bufs=2, space="PSUM"))
        dpool = ctx.enter_context(tc.tile_pool(name="dn", bufs=2, space="PSUM"))
        jpool = ctx.enter_context(tc.tile_pool(name="jk", bufs=1, space="PSUM"))

        ones = onepool.tile([128, 1], f32)
        nc.vector.memset(ones[:], 1.0)
        ones_bf = onepool.tile([128, 1], bf16)
        nc.vector.memset(ones_bf[:], 1.0)
        warm = onepool.tile([128, 256], bf16)
        nc.vector.memset(warm[:], 0.0)
        jk = jpool.tile([1, 256], f32)

        tails = []  # (acc_lo, acc_hi, denom, b) pending finalization

        def emit_tail():
            acc_lo, acc_hi, denom, bb = tails.pop(0)
            recip = opool.tile([1, 1], f32, name="recip")
            nc.vector.reciprocal(recip[:], denom[:])
            orow = opool.tile([1, D], f32, name="orow")
            nc.scalar.activation(out=orow[:, 0:512], in_=acc_lo[:],
                                 func=mybir.ActivationFunctionType.Copy,
                                 scale=recip[:])
            nc.scalar.activation(out=orow[:, 512:1024], in_=acc_hi[:],
                                 func=mybir.ActivationFunctionType.Copy,
                                 scale=recip[:])
            nc.sync.dma_start(out=out[bb : bb + 1, :], in_=orow[:])

        for b in range(bl):
            qb = qpool.tile([128, D], bf16)
            nc.sync.dma_start(
                out=qb[:],
                in_=q[b : b + 1, :].broadcast_to([128, D]))

            scols = scpool.tile([128, NCH * RPP], f32)
            ecols = ecpool.tile([128, NCH * RPP], bf16)
            ered = erpool.tile([128, 1], f32)
            acc_lo = plo.tile([1, 512], f32)
            acc_hi = phi.tile([1, 512], f32)
            cts = []

            for h in range(NCH):
                a = A_PER_CHUNK[h % 2]
                pm = POOL_MULT
                ct = cpool.tile([128, RPP, D], bf16, name="ct")
                dma_eng = nc.sync if h % 2 == 0 else nc.scalar
                dma_eng.dma_start(out=ct[:], in_=c[b, h])
                cts.append(ct)

                prod = prpool.tile([128, a, D], bf16, name="prod")
                if a - pm:
                    nc.vector.tensor_tensor(
                        out=prod[:, 0 : a - pm], in0=ct[:, 0 : a - pm],
                        in1=qb[:].unsqueeze(1).broadcast_to([128, a - pm, D]),
                        op=mybir.AluOpType.mult)
                for j in range(a - pm, a):
                    nc.gpsimd.tensor_mul(prod[:, j], ct[:, j], qb[:])

                for j in range(RPP):
                    col = h * RPP + j
                    acc = scols[:, col : col + 1]
                    if j < a:
                        sl = prod[:, j]
                        nc.scalar.activation(
                            out=sl, in_=sl,
                            func=mybir.ActivationFunctionType.Copy,
                            accum_out=acc,
                        )
                    else:
                        scr = stpool.tile([128, D], bf16, name="scr")
                        nc.vector.scalar_tensor_tensor(
                            out=scr[:],
                            in0=ct[:, j], scalar=1.0, in1=qb[:],
                            op0=mybir.AluOpType.mult,
                            op1=mybir.AluOpType.mult,
                            accum_out=acc,
                        )

                # PE keep-warm: dummy matmuls gated on this chunk's score
                # columns fire spread across the PE's inter-sample gap,
                # stopping the HAM activity monitor from re-throttling.
                for col in (h * RPP + 2, h * RPP + 4, h * RPP + 6):
                    nc.tensor.matmul(jk[:, 0:1], ones[:],
                                     scols[:, col : col + 1],
                                     start=True, stop=True,
                                     skip_group_check=True)

            nc.scalar.activation(
                out=ecols[:], in_=scols[:],
                func=mybir.ActivationFunctionType.Exp,
                accum_out=ered[:],
            )

            # weighted-sum matmuls, grouped by PSUM bank
            for acc, lo in ((acc_lo, 0), (acc_hi, 512)):
                for h in range(NCH):
                    for j in range(RPP):
                        seq = h * RPP + j
                        e = ecols[:, seq : seq + 1]
                        nc.tensor.matmul(acc[:], e,
                                         cts[h][:, j, lo : lo + 512],
                                         start=(seq == 0),
                                         stop=(seq == NCH * RPP - 1))

            denom = dpool.tile([1, 1], f32)
            nc.tensor.matmul(denom[:], ones[:], ered[:], start=True, stop=True)

            tails.append((acc_lo, acc_hi, denom, b))
            if len(tails) > 1:
                emit_tail()

        while tails:
            emit_tail()

    nc.compile()
    return nc


def _run(query, concept, trace=False, trace_kwargs=None):
    if "nc" not in _cache:
        _cache["nc"] = build_nc()
    nc = _cache["nc"]

    bf = ml_dtypes.bfloat16
    qs = (np.asarray(query, np.float32) * SCALE).astype(bf)
    cb = np.asarray(concept, np.float32).astype(bf)

    in_maps = []
    for i in range(NCORES):
        in_maps.append({
            "query": np.ascontiguousarray(qs[i * BL : (i + 1) * BL]),
            "concept": np.ascontiguousarray(
                cb[i * BL : (i + 1) * BL].reshape(BL, NCH, 128, RPP, D)),
        })
    res = run_bass_kernel_spmd(
        nc, in_maps, core_ids=list(range(NCORES)),
        trace=trace, **(trace_kwargs or {}),
    )
    out = np.concatenate([res.results[i]["out"] for i in range(NCORES)], axis=0)
    return out.astype(np.float32), res


def kernel(query: np.ndarray, concept: np.ndarray) -> np.ndarray:
    out, _ = _run(np.asarray(query, np.float32), np.asarray(concept, np.float32))
    return out
